# revision 27
# baseline (speedup 1.0000x reference)
"""GCN actor kernel for 8 TRN2 NeuronCores (Bass/Tile).

Math (mirrors the reference):
    deg[v]  = in-degree(v) + 1 (self loop);  dinv = deg^-1/2
    y[v]    = dinv[v] * (x[v] @ conv_w)              (dinv folded into x on host)
    acc[v]  = sum_{(s,v) in E+selfloops} y[s]         (segment sum)
    h[v]    = relu(dinv[v] * acc[v] + conv_b)
    z[v]    = (h[v] - mean(h[v])) * rsqrt(var(h[v]) + eps)      (LayerNorm core)
    pooled  = allreduce(sum_v z[v]) * ln_g + N * ln_b  (pad rows give z == 0)
    out     = tanh(relu(pooled @ w2 + b2) @ w3 + b3)

Sharding: nodes (and their incoming edges) are dst-sharded across the 8
cores; the y table (bf16) is computed replicated on every core so gathers
are local.  Only the [128] pooled vector is all-reduced.

Segment summation is done WITHOUT dma_scatter_add (whose CCE
read-modify-write races on duplicate destinations): edges are sorted by
destination and tiled into 128-edge tiles whose destinations lie in one
aligned 32-node window.  Each tile's messages are gathered (dma_gather)
as a [128 tokens, 128 feat] bf16 tile; a [128, 32] one-hot selection
matrix S (built on DVE by comparing the per-token "dstrel" value against
iota 0..31) folds the tile into its window's 32 rows of a per-node-tile
PSUM accumulator via one TensorE matmul.  Pad tokens carry dstrel = -1,
matching no window slot, so they contribute nothing.  The tiling (tiles
per window, chunk layout) is made uniform across cores (max over cores)
so the SPMD graph is identical; only idx/dstrel DATA differs per core.

SWDGE idx layout (probed on HW): idx i of an instruction lives at
[i % 16, i // 16] in an int16 SBUF tile, and rows 0-15 must be
REPLICATED into rows 16-31 (RX Q7 core reads partitions 0-15, TX core
reads 16-31).  dma_gather / dma_scatter_add are capped at 1024 tokens
per instruction (idx-streamer limit) - hence 8-tile gather chunks.
"""

import numpy as np
import ml_dtypes

import concourse.bass as bass
import concourse.bacc as bacc
import concourse.tile as tile
import concourse.mybir as mybir
from concourse.bass_utils import run_bass_kernel_spmd
from concourse.masks import make_identity

F32 = mybir.dt.float32
BF16 = mybir.dt.bfloat16
I16 = mybir.dt.int16
NPBF = ml_dtypes.bfloat16

NCORES = 8
D = 128          # feature dim (D_IN == D_H == 128)
DA = 64          # action dim
LN_EPS = 1e-5
W = 128          # dst window per edge tile (aligned, = node tile)
CHT = 8          # tiles per gather chunk (8*128 = 1024 token cap)


def _round_up(a, b):
    return -(-a // b) * b


def build_graph(cfg):
    """Build + compile the SPMD Bass graph. cfg keys:
    NPAD, HALF, tiles_a/tiles_b (tuple of window ids per 128-edge tile,
    uniform across cores), debug, single."""
    NPAD = cfg["NPAD"]
    HALF = cfg["HALF"]
    tiles = [cfg["tiles_a"], cfg["tiles_b"]]
    NPC = NPAD // NCORES
    NTO = NPC // 128
    NT = NPAD // 128

    nta, ntb = len(tiles[0]), len(tiles[1])
    LA16, LB16 = nta * 8, ntb * 8    # idx cols (128 tokens = 8 cols of 16)

    nc = bacc.Bacc(
        "TRN2",
        target_bir_lowering=False,
        debug=cfg.get("debug", False),
        num_devices=NCORES,
    )

    xa = nc.dram_tensor("xa", [min(HALF, NPAD), D], BF16, kind="ExternalInput")
    if NPAD > HALF:
        xb = nc.dram_tensor("xb", [NPAD - HALF, D], BF16, kind="ExternalInput")
    cw = nc.dram_tensor("cw", [D, D], F32, kind="ExternalInput")
    srca = nc.dram_tensor("srca", [128, max(LA16, 1)], I16, kind="ExternalInput")
    dra = nc.dram_tensor("dra", [128, max(nta, 1)], F32, kind="ExternalInput")
    if ntb:
        srcb = nc.dram_tensor("srcb", [128, LB16], I16, kind="ExternalInput")
        drb = nc.dram_tensor("drb", [128, ntb], F32, kind="ExternalInput")
    iotaw = nc.dram_tensor("iotaw", [CHT * W], BF16, kind="ExternalInput")
    dinvo = nc.dram_tensor("dinvo", [128, NTO], F32, kind="ExternalInput")
    cb = nc.dram_tensor("cb", [D], F32, kind="ExternalInput")
    gcol = nc.dram_tensor("gcol", [D], F32, kind="ExternalInput")
    lbs = nc.dram_tensor("lbs", [D], F32, kind="ExternalInput")  # ln_b * N
    w2 = nc.dram_tensor("w2", [D, D], F32, kind="ExternalInput")
    b2 = nc.dram_tensor("b2", [D], F32, kind="ExternalInput")
    w3 = nc.dram_tensor("w3", [D, DA], F32, kind="ExternalInput")
    b3 = nc.dram_tensor("b3", [DA], F32, kind="ExternalInput")
    out_ext = nc.dram_tensor("out", [DA, 1], F32, kind="ExternalOutput")

    cc_in = nc.dram_tensor("cc_in", [D, 1], F32)
    cc_out = nc.dram_tensor("cc_out", [D, 1], F32, addr_space="Shared")

    with tile.TileContext(nc) as tc:
        with tc.tile_pool(name="persist", bufs=1) as per:
            cw_t = per.tile([D, D], F32)
            nc.sync.dma_start(out=cw_t[:], in_=cw[:, :])
            ident = per.tile([128, 128], F32)
            make_identity(nc, ident[:])
            # HW only reads idx partitions 0-31 (RX/TX Q7 cores); CoreSim
            # asserts all 128 rows, so load the full tile only in debug
            IDXR = 128 if cfg.get("debug") else 32
            sa_t = per.tile([128, max(LA16, 1)], I16)
            nc.sync.dma_start(out=sa_t[:IDXR, :], in_=srca[:IDXR, :])
            da_t = per.tile([128, max(nta, 1)], F32)
            nc.sync.dma_start(out=da_t[:], in_=dra[:, :])
            if ntb:
                sb_t = per.tile([128, LB16], I16)
                nc.sync.dma_start(out=sb_t[:IDXR, :], in_=srcb[:IDXR, :])
                db_t = per.tile([128, ntb], F32)
                nc.sync.dma_start(out=db_t[:], in_=drb[:, :])
            iw_t = per.tile([128, W], BF16)
            nc.sync.dma_start(
                out=iw_t[:],
                in_=bass.AP(tensor=iotaw, offset=0, ap=[[0, 128], [1, W]]),
            )
            dinvo_t = per.tile([128, NTO], F32)
            nc.sync.dma_start(out=dinvo_t[:], in_=dinvo[:, :])
            cb_t = per.tile([128, D], F32)
            nc.sync.dma_start(
                out=cb_t[:], in_=bass.AP(tensor=cb, offset=0, ap=[[0, 128], [1, D]])
            )
            eps_t = per.tile([128, 1], F32)
            nc.vector.memset(eps_t[:], LN_EPS)
            pool_t = per.tile([128, D], F32)
            nc.vector.memset(pool_t[:], 0.0)

            # ---------------- phase B+C: gather -> S-matmul -> LN/pool ------
            # stream state: (tile windows, idx tile, dstrel tile, gather src)
            st = [
                dict(tiles=tiles[0], it=sa_t, dt=da_t, src=xa[:, :], chunks={}),
                dict(
                    tiles=tiles[1],
                    it=sb_t if ntb else None,
                    dt=db_t if ntb else None,
                    src=xb[:, :] if ntb else None,
                    chunks={},
                ),
            ]
            # tiles grouped per node tile j (window w -> j = w*W//128)
            tiles_by_node = [[[] for _ in range(NTO)] for _ in range(2)]
            for s in range(2):
                for t, w in enumerate(st[s]["tiles"]):
                    tiles_by_node[s][w * W // 128].append(t)

            with (
                tc.tile_pool(name="gbuf", bufs=6) as gbp,
                tc.tile_pool(name="sbuf_s", bufs=6) as sbp,
                tc.tile_pool(name="bps", bufs=4, space="PSUM") as bps,
                tc.tile_pool(name="bpt", bufs=1, space="PSUM") as bpt,
                tc.tile_pool(name="bph", bufs=2, space="PSUM") as bph,
                tc.tile_pool(name="chh", bufs=3) as chh,
                tc.tile_pool(name="cst", bufs=6) as cst,
                tc.tile_pool(name="cps", bufs=1, space="PSUM") as cps,
            ):

                def ensure_chunk(s, c):
                    S = st[s]
                    if c in S["chunks"]:
                        return S["chunks"][c]
                    ntiles = len(S["tiles"])
                    ntc = min(CHT, ntiles - c * CHT)
                    ntok = ntc * 128
                    g = gbp.tile([128, CHT, D], BF16, tag=f"g{s}")
                    nc.gpsimd.dma_gather(
                        g[:, :ntc, :],
                        S["src"],
                        S["it"][:, c * CHT * 8 : (c * CHT + ntc) * 8],
                        ntok,
                        ntok,
                        D,
                    )
                    sm = sbp.tile([128, CHT, W], BF16, tag=f"s{s}")
                    for k in range(ntc):
                        nc.vector.tensor_scalar(
                            out=sm[:, k, :],
                            in0=iw_t[:],
                            scalar1=S["dt"][:, c * CHT + k : c * CHT + k + 1],
                            scalar2=None,
                            op0=mybir.AluOpType.is_equal,
                        )
                    S["chunks"][c] = (g, sm)
                    return S["chunks"][c]

                for j in range(NTO):
                    ps = bps.tile([128, 128], F32, tag="ps")
                    nc.vector.memset(ps[:], 0.0)
                    mms = [(s, t) for s in range(2) for t in tiles_by_node[s][j]]
                    for i, (s, t) in enumerate(mms):
                        g, sm = ensure_chunk(s, t // CHT)
                        k = t % CHT
                        off = (st[s]["tiles"][t] * W) % 128
                        nc.tensor.matmul(
                            ps[off : off + W, :],
                            lhsT=sm[:, k, :],
                            rhs=g[:, k, :],
                            start=False,
                            stop=(i == len(mms) - 1),
                            skip_group_check=True,
                            tile_position=(0, off),
                        )
                    # acc_x -> (acc_x @ conv_w): evict, transpose, matmul
                    axs = chh.tile([128, D], F32, tag="axs")
                    nc.scalar.activation(
                        out=axs[:], in_=ps[:],
                        func=mybir.ActivationFunctionType.Copy,
                    )
                    pst = bpt.tile([128, 128], F32)
                    nc.tensor.transpose(out=pst[:], in_=axs[:], identity=ident[:])
                    axT = chh.tile([128, D], F32, tag="axT")
                    nc.vector.tensor_copy(out=axT[:], in_=pst[:])
                    hps = bph.tile([128, D], F32)
                    nc.tensor.matmul(
                        hps[:], lhsT=axT[:], rhs=cw_t[:], start=True, stop=True
                    )
                    # epilogue: h = relu(dinv * acc + cb); LN core; z-pool
                    h = chh.tile([128, D], F32)
                    nc.vector.tensor_scalar_mul(
                        out=h[:], in0=hps[:], scalar1=dinvo_t[:, j : j + 1]
                    )
                    nc.vector.tensor_add(out=h[:], in0=h[:], in1=cb_t[:])
                    nc.scalar.activation(
                        out=h[:], in_=h[:], func=mybir.ActivationFunctionType.Relu
                    )
                    stt = cst.tile([128, nc.vector.BN_STATS_DIM], F32)
                    nc.vector.bn_stats(out=stt[:], in_=h[:])
                    mv = cst.tile([128, nc.vector.BN_AGGR_DIM], F32)
                    nc.vector.bn_aggr(out=mv[:], in_=stt[:])
                    sq = cst.tile([128, 1], F32)
                    nc.scalar.activation(
                        out=sq[:],
                        in_=mv[:, 1:2],
                        func=mybir.ActivationFunctionType.Sqrt,
                        bias=eps_t[:],
                    )
                    rstd = cst.tile([128, 1], F32)
                    nc.vector.reciprocal(out=rstd[:], in_=sq[:])
                    nc.vector.tensor_scalar(
                        out=h[:],
                        in0=h[:],
                        scalar1=mv[:, 0:1],
                        scalar2=rstd[:],
                        op0=mybir.AluOpType.subtract,
                        op1=mybir.AluOpType.mult,
                    )
                    nc.vector.tensor_add(out=pool_t[:], in0=pool_t[:], in1=h[:])

                # transpose pool_t -> [feature, 1] column
                pps = cps.tile([128, 128], F32, tag="tail")
                nc.tensor.transpose(out=pps[:], in_=pool_t[:], identity=ident[:])
                tp = chh.tile([128, 128], F32)
                nc.vector.tensor_copy(out=tp[:], in_=pps[:])
                zsum = per.tile([128, 1], F32)
                nc.vector.tensor_reduce(
                    out=zsum[:],
                    in_=tp[:],
                    axis=mybir.AxisListType.X,
                    op=mybir.AluOpType.add,
                )

                # ---------------- phase D: all-reduce + MLP ----------------
                nc.sync.dma_start(out=cc_in[:, :], in_=zsum[:])
                if cfg.get("single"):
                    nc.sync.dma_start(out=cc_out[:, :], in_=cc_in[:, :])
                else:
                    nc.gpsimd.collective_compute(
                        "AllReduce",
                        mybir.AluOpType.add,
                        replica_groups=[list(range(NCORES))],
                        ins=[cc_in.ap().opt()],
                        outs=[cc_out.ap().opt()],
                    )
                pooled = per.tile([128, 1], F32)
                nc.sync.dma_start(out=pooled[:], in_=cc_out[:, :])
                g_t = per.tile([128, 1], F32)
                nc.sync.dma_start(out=g_t[:], in_=gcol[:, None])
                lb_t = per.tile([128, 1], F32)
                nc.sync.dma_start(out=lb_t[:], in_=lbs[:, None])
                nc.vector.tensor_mul(out=pooled[:], in0=pooled[:], in1=g_t[:])
                nc.vector.tensor_add(out=pooled[:], in0=pooled[:], in1=lb_t[:])

                w2_t = per.tile([D, D], F32)
                nc.sync.dma_start(out=w2_t[:], in_=w2[:, :])
                b2_t = per.tile([D, 1], F32)
                nc.sync.dma_start(out=b2_t[:], in_=b2[:, None])
                w3_t = per.tile([D, DA], F32)
                nc.sync.dma_start(out=w3_t[:], in_=w3[:, :])
                b3_t = per.tile([DA, 1], F32)
                nc.sync.dma_start(out=b3_t[:], in_=b3[:, None])

                ps2 = cps.tile([D, 1], F32, tag="tail")
                nc.tensor.matmul(ps2[:], lhsT=w2_t[:], rhs=pooled[:], start=True, stop=True)
                a_t = per.tile([D, 1], F32)
                nc.scalar.activation(
                    out=a_t[:],
                    in_=ps2[:],
                    func=mybir.ActivationFunctionType.Relu,
                    bias=b2_t[:],
                )
                ps3 = cps.tile([DA, 1], F32, tag="tail")
                nc.tensor.matmul(ps3[:], lhsT=w3_t[:], rhs=a_t[:], start=True, stop=True)
                o_t = per.tile([DA, 1], F32)
                nc.scalar.activation(
                    out=o_t[:],
                    in_=ps3[:],
                    func=mybir.ActivationFunctionType.Tanh,
                    bias=b3_t[:],
                )
                nc.sync.dma_start(out=out_ext[:, :], in_=o_t[:])

    nc.compile()
    return nc


def _wrap16(a):
    """Pack a (multiple-of-128)-length idx vector into the SWDGE int16
    layout: idx i at [i % 16, i // 16], replicated into rows 16-31."""
    L = len(a)
    w = np.zeros((128, max(L // 16, 1)), np.int16)
    if L:
        w16 = np.ascontiguousarray(a.reshape(L // 16, 16).T)
        w[0:16] = w16
        w[16:32] = w16
    return w


def _pack_stream(s_list, d_list, NPC):
    """dst-sorted, window-tiled token layout, uniform tiling across cores.
    Returns (tiles, src_arrays, dstrel_arrays): tiles is a tuple of window
    ids (one per 128-edge tile); arrays are per-core."""
    ncores = len(s_list)
    nw = NPC // W
    cnt = np.zeros((ncores, nw), np.int64)
    srt = []
    for c in range(ncores):
        order = np.argsort(d_list[c], kind="stable")
        ds = d_list[c][order]
        ss = s_list[c][order]
        cnt[c] = np.bincount(ds // W, minlength=nw)
        srt.append((ss, ds))
    tiles_w = -(-cnt.max(axis=0) // 128)          # tiles per window
    tiles = np.repeat(np.arange(nw), tiles_w)     # window id per tile
    ntiles = len(tiles)
    L = ntiles * 128
    tile_base = np.zeros(nw + 1, np.int64)
    np.cumsum(tiles_w, out=tile_base[1:])

    src_arrays, dr_arrays = [], []
    for c in range(ncores):
        ss, ds = srt[c]
        src_tok = np.zeros(L, np.int64)
        dr_tok = np.full(L, -1.0, np.float32)
        if len(ds):
            wins = ds // W
            wstart = np.r_[0, np.cumsum(cnt[c])][wins]
            rank = np.arange(len(ds)) - wstart
            pos = tile_base[wins] * 128 + rank
            src_tok[pos] = ss
            dr_tok[pos] = (ds - wins * W).astype(np.float32)
        src_arrays.append(_wrap16(src_tok.astype(np.int16)))
        if ntiles:
            dr_arrays.append(
                np.ascontiguousarray(dr_tok.reshape(ntiles, 128).T)
            )
        else:
            dr_arrays.append(np.zeros((128, 1), np.float32))
    return tuple(int(w_) for w_ in tiles), src_arrays, dr_arrays


def prep(x, edge_index, conv_w, conv_b, ln_g, ln_b, w2, b2, w3, b3, HALF):
    """Host-side sharding. Returns (cfg, in_maps)."""
    x = np.asarray(x, np.float32)
    ei = np.asarray(edge_index).astype(np.int64)
    conv_w = np.asarray(conv_w, np.float32)
    conv_b = np.asarray(conv_b, np.float32)
    ln_g = np.asarray(ln_g, np.float32)
    ln_b = np.asarray(ln_b, np.float32)
    w2 = np.asarray(w2, np.float32)
    b2 = np.asarray(b2, np.float32)
    w3 = np.asarray(w3, np.float32)
    b3 = np.asarray(b3, np.float32)

    N, Din = x.shape
    NPAD = _round_up(N, 1024)
    NPC = NPAD // NCORES
    NTO = NPC // 128

    src, dst = ei[0], ei[1]
    deg = np.bincount(dst, minlength=N).astype(np.float64) + 1.0
    dinv = 1.0 / np.sqrt(deg)

    xs = (x.astype(np.float64) * dinv[:, None]).astype(np.float32)
    xrow = np.zeros((NPAD, Din), NPBF)
    xrow[:N] = xs.astype(NPBF)
    dinv = dinv.astype(np.float32)

    core = dst // NPC
    sA, dA, sB, dB = [], [], [], []
    for c in range(NCORES):
        m = core == c
        s = src[m]
        d = dst[m] - c * NPC
        vown = np.arange(c * NPC, min((c + 1) * NPC, N), dtype=np.int64)
        s = np.concatenate([s, vown])
        d = np.concatenate([d, vown - c * NPC])
        a = s < HALF
        sA.append(s[a])
        dA.append(d[a])
        sB.append(s[~a] - HALF)
        dB.append(d[~a])

    # Per-core node-tile permutation: the pooled output is invariant to
    # node-tile order, so each core privately rank-orders its node tiles
    # by edge count. High-count tiles then rank-align across cores, which
    # tightens the max-over-cores in the uniform tiling (less gather pad).
    # dinvo columns are permuted to match (pure per-core data).
    perms = []
    for c in range(NCORES):
        tot = np.bincount(dA[c] // 128, minlength=NTO) + np.bincount(
            dB[c] // 128, minlength=NTO
        )
        perm = np.argsort(-tot, kind="stable")  # rank r -> physical tile
        perms.append(perm)
        rank_of = np.empty(NTO, np.int64)
        rank_of[perm] = np.arange(NTO)
        dA[c] = rank_of[dA[c] // 128] * 128 + dA[c] % 128
        dB[c] = rank_of[dB[c] // 128] * 128 + dB[c] % 128

    tiles_a, sa_arr, da_arr = _pack_stream(sA, dA, NPC)
    tiles_b, sb_arr, db_arr = _pack_stream(sB, dB, NPC)

    cfg = {"NPAD": NPAD, "HALF": HALF, "tiles_a": tiles_a, "tiles_b": tiles_b}

    iw = np.tile(np.arange(W, dtype=np.float32), CHT).astype(NPBF)
    in_maps = []
    for c in range(NCORES):
        m = {
            "xa": xrow[: min(HALF, NPAD)],
            "cw": conv_w,
            "srca": sa_arr[c],
            "dra": da_arr[c],
            "iotaw": iw,
            "cb": conv_b,
            "gcol": ln_g,
            "lbs": (ln_b * float(N)).astype(np.float32),
            "w2": w2,
            "b2": b2,
            "w3": w3,
            "b3": b3,
        }
        if NPAD > HALF:
            m["xb"] = xrow[HALF:]
        if len(tiles_b):
            m["srcb"] = sb_arr[c]
            m["drb"] = db_arr[c]
        dpad = np.zeros(NPC, np.float32)
        cnt = max(0, min((c + 1) * NPC, N) - c * NPC)
        dpad[:cnt] = dinv[c * NPC : c * NPC + cnt]
        m["dinvo"] = np.ascontiguousarray(dpad.reshape(NTO, 128)[perms[c]].T)
        in_maps.append(m)
    return cfg, in_maps


_CACHE = {}


def kernel(**inputs):
    HALF = 32768
    cfg, in_maps = prep(
        inputs["x"],
        inputs["edge_index"],
        inputs["conv_w"],
        inputs["conv_b"],
        inputs["ln_g"],
        inputs["ln_b"],
        inputs["w2"],
        inputs["b2"],
        inputs["w3"],
        inputs["b3"],
        HALF,
    )
    key = (cfg["NPAD"], cfg["HALF"], cfg["tiles_a"], cfg["tiles_b"])
    if key not in _CACHE:
        _CACHE[key] = build_graph(cfg)
    nc = _CACHE[key]
    res = run_bass_kernel_spmd(nc, in_maps, core_ids=list(range(NCORES)))
    return np.ascontiguousarray(
        res.results[0]["out"].astype(np.float32).reshape(1, DA)
    )
